# revision 26
# baseline (speedup 1.0000x reference)
"""CapsuleNetwork forward on 8 Trainium2 NeuronCores (Bass/Tile).

Math (validated against the jax reference in a numpy prototype):
  conv+relu:  h = relu(conv2d(x, conv_w) + conv_b)            [64,32,20,20]
  stage 2:    u1 = einsum('jkmc,bk->bjkm', W1, h.flat)  and routing(u1, 1)
              collapses (softmax of zeros is uniform 1/8) to
                s[b,j,m] = (1/8) * sum_k h.flat[b,k] * sum_c W1[j,k,m,c]
              i.e. ONE matmul contracting k, with the c-reduction done in
              PSUM columns and folded by a vector reduce at the end.
  v1 = squash(s);  u2 = einsum('jkmc,bkc->bjkm', W2, v1);  v2 = routing(u2, 3)

Sharding: the W1 stream dominates (memory regime), so we shard the
contraction k = (ch, y, x') by conv CHANNEL: core i owns channels 4i..4i+3,
computes its 4-channel slice of the conv and the partial s over its 1600 k's
(every element of W1 read exactly once chip-wide).  W1 is streamed in bf16
(weight-only quantization; whole-pipeline rel err ~5e-3 vs the 2e-2 gate),
which halves the HBM traffic that bounds this kernel.  Partials [128,32] are
gathered and summed on host (the unshard step), then a small phase-B kernel
on core 0 runs squash -> digit-caps matmul -> 3-iter routing.  (A fused
single-kernel variant with an on-device AllReduce was measured: the 8-core
launch skew under this runner is ~60us, so any cross-core sync inside the
kernel inflates the measured span.  Two independent launches win.)

The conv is expressed as 2 stationary banded-weight matmuls so its output
lands directly in the [k-on-partitions, batch] layout stage 2 needs.
Host prep is layout only (transpose/slice/band-expansion/dtype of weights).
"""

import contextlib
import ctypes
import os
import sys
import types

os.environ.setdefault("NEURON_RT_RESET_CORES", "1")  # recover wedged cores


def _install_axon_ntff_shim():
    """concourse.bass_utils imports antenv.axon_hooks for trace=True under
    axon; this image's antenv lacks that module. Recreate the documented
    ctypes hook (see trn_agent_boot) so tracing works instead of crashing."""
    try:
        import antenv.axon_hooks  # noqa: F401
        return
    except ImportError:
        pass

    def _make_hook():
        so_path = "/opt/axon/libaxon_pjrt.so"
        if not os.path.exists(so_path):
            return None
        lib = ctypes.CDLL(so_path)
        if not hasattr(lib, "axon_start_nrt_profile"):
            return None
        lib.axon_start_nrt_profile.argtypes = [
            ctypes.POINTER(ctypes.c_int64), ctypes.c_size_t]
        lib.axon_start_nrt_profile.restype = ctypes.c_int64
        lib.axon_stop_nrt_profile.argtypes = [ctypes.c_char_p]
        lib.axon_stop_nrt_profile.restype = ctypes.c_int64

        @contextlib.contextmanager
        def _hook(output_dir, device_ids):
            import jax
            jax.devices()
            if device_ids:
                ids = (ctypes.c_int64 * len(device_ids))(*device_ids)
                rc = lib.axon_start_nrt_profile(ids, len(device_ids))
            else:
                rc = lib.axon_start_nrt_profile(None, 0)
            if rc != 0:
                raise RuntimeError(f"axon_start_nrt_profile rc={rc}")
            try:
                yield
            finally:
                n = lib.axon_stop_nrt_profile(str(output_dir).encode())
                print(f"profile: {n} file(s) written to {output_dir}",
                      file=sys.stderr)

        return _hook

    mod = types.ModuleType("antenv.axon_hooks")
    hook = _make_hook()
    mod.get_axon_ntff_profile_hook = lambda: hook
    mod.set_axon_ntff_profile_hook = lambda h: None
    sys.modules["antenv.axon_hooks"] = mod


_install_axon_ntff_shim()

import ml_dtypes
import numpy as np

import concourse.bacc as bacc
import concourse.bass as bass
import concourse.tile as tile
from concourse import mybir
from concourse.bass_utils import run_bass_kernel_spmd

F32 = mybir.dt.float32
BF16 = mybir.dt.bfloat16
NPBF = ml_dtypes.bfloat16
AX = mybir.AxisListType
AF = mybir.ActivationFunctionType
ALU = mybir.AluOpType

B = 64          # batch
NCORES = 8
NCH = 4         # conv channels per core
P1 = 126        # conv contraction tile (2 tiles cover the 9x28 input window)
Q = NCH * 20    # 80 = (ch, x') partitions per core
J1, M1, C1 = 8, 8, 32
J2, K2, M2, C2 = 10, 8, 16, 8
JM = J1 * M1    # 64
JKM = J2 * K2 * M2  # 1280
NROW = 20 * Q   # 1600 k-rows per core
NBLK = 13       # ceil(1600 / 128) stationary k-blocks

_CACHE = {}

# ----------------------------------------------------------------------------
# host-side relayout helpers (layout + dtype only, no model arithmetic)
# ----------------------------------------------------------------------------

def _prep_xwin(x):
    """xwin[p, y, t, b] = xT[28y + 126t + p, b]: the two 126-row K-tiles of
    the 9-row input window for each conv output row y, partition-major and
    y-second so each conv y-chunk loads as one contiguous DMA slice."""
    xT = np.ascontiguousarray(x.reshape(B, 784).T)            # [pix, b]
    p = np.arange(P1)[:, None, None]
    y = np.arange(20)[None, :, None]
    t = np.arange(2)[None, None, :]
    rows = 28 * y + P1 * t + p                                # [126,20,2]
    return np.ascontiguousarray(xT[rows].astype(NPBF))        # [126,20,2,64]


def _prep_wband(conv_w, ch_lo):
    """wband[p, t, (ch,x')] = conv_w[ch_lo+ch, 0, dy, xin-x'] / 8
    where (dy, xin) = divmod(126t + p, 28).  The 1/8 is the uniform
    softmax coupling of routing(u1, 1), folded into the (linear) conv;
    relu(z/8) == relu(z)/8."""
    wb = np.zeros((252, NCH, 20), np.float32)
    cw = conv_w[ch_lo:ch_lo + NCH, 0]                         # [4, 9, 9]
    for idx in range(252):
        dy, xin = divmod(idx, 28)
        for xp in range(max(0, xin - 8), min(20, xin + 1)):
            wb[idx, :, xp] = cw[:, dy, xin - xp] * 0.125
    return np.ascontiguousarray(
        wb.reshape(2, P1, Q).transpose(1, 0, 2).astype(NPBF))  # [126,2,80]


def _prep_w1r(W1, ch_lo):
    """w1r[(y, ch, x'), (j, m, c)] — this core's k-rows of W1 in bf16,
    zero-padded 1600 -> 1664 rows so the stream is 13 full 128-partition
    tiles (zero rows multiply against garbage xk pad rows, contributing 0)."""
    v = W1.reshape(J1, 32, 20, 20, M1 * C1)[:, ch_lo:ch_lo + NCH]
    v = v.transpose(2, 1, 3, 0, 4).reshape(NROW, J1 * M1 * C1)
    out = np.zeros((NBLK * 128, J1 * M1 * C1), NPBF)
    out[:NROW] = v.astype(NPBF)
    return out


def _prep_w2bd(W2):
    """Block-diagonal digit-caps weights, output columns ordered (k, m, j):
    bd[(k',c), (k,m,j)] = W2[j,k,m,c] iff k == k', so u2 = v1_flat @ bd in
    one matmul.  The (k,m,j) order makes every big routing op walk with a
    real stride-1 innermost dim: k-reductions and m-reductions become
    halving adds over contiguous slabs, and the c/v broadcasts put their
    0-stride dim outermost (0-stride innermost measured 1.8x slower)."""
    bd5 = np.zeros((K2, C2, K2, M2, J2), np.float32)
    for k in range(K2):
        bd5[k, :, k, :, :] = W2[:, k, :, :].transpose(2, 1, 0)
    return np.ascontiguousarray(bd5.reshape(K2 * C2, JKM).astype(NPBF))


# ----------------------------------------------------------------------------
# phase A: per-core conv + partial capsule matmul  (SPMD on 8 cores)
# ----------------------------------------------------------------------------

def _build_phase_a():
    nc = bacc.Bacc("TRN2", target_bir_lowering=False, debug=False,
                   num_devices=NCORES)
    xwin_d = nc.dram_tensor("xwin", [P1, 20, 2, B], BF16, kind="ExternalInput")
    wband_d = nc.dram_tensor("wband", [P1, 2, Q], BF16, kind="ExternalInput")
    bias_d = nc.dram_tensor("bias", [Q, 1], F32, kind="ExternalInput")
    w1r_d = nc.dram_tensor("w1r", [NBLK * 128, J1 * M1 * C1], BF16,
                           kind="ExternalInput")
    sp_d = nc.dram_tensor("sp", [128, 32], F32, kind="ExternalOutput")

    with tile.TileContext(nc) as tc:
        with (
            tc.tile_pool(name="const", bufs=1) as const,
            tc.tile_pool(name="w1pool", bufs=1) as w1pool,
            tc.tile_pool(name="xkpool", bufs=1) as xkpool,
            tc.tile_pool(name="cpsum", bufs=1, space="PSUM") as cpsum_pool,
            tc.tile_pool(name="spsum", bufs=1, space="PSUM") as spsum_pool,
        ):
            # ALL HBM traffic rides ONE queue (sync): xw chunk 1, conv
            # weights, then the rest of xw, then the four big W1 chunks.
            # Concurrent DMA queues do NOT share SDMA bandwidth fairly (a
            # megabyte-transfer queue starves a small one ~8:1, measured), so
            # FIFO order on one queue is the only reliable way to get the
            # conv inputs in early; total bytes -- and the stream-end time --
            # are unchanged.
            xw = const.tile([P1, 20, 2, B], BF16)
            nc.sync.dma_start(out=xw[:, 0:8, :, :], in_=xwin_d[:, 0:8, :, :])
            wb = const.tile([P1, 2, Q], BF16)
            nc.sync.dma_start(out=wb[:], in_=wband_d[:])
            bias_t = const.tile([Q, 1], F32)
            nc.sync.dma_start(out=bias_t[:], in_=bias_d[:])
            nc.sync.dma_start(out=xw[:, 8:16, :, :], in_=xwin_d[:, 8:16, :, :])
            nc.sync.dma_start(out=xw[:, 16:20, :, :],
                              in_=xwin_d[:, 16:20, :, :])

            # W1 stream: one [128, 13*4KB] tile fed by 4 big DMAs (1.6-2 MB
            # each -- small transfers sit below the SDMA efficiency knee).
            # Stage-2 matmuls wait on sub-region sems, chasing chunk by chunk.
            w1all = w1pool.tile([128, NBLK, J1 * M1 * C1], BF16)
            for lo, hi in ((0, 4), (4, 8), (8, 12), (12, 13)):
                nc.sync.dma_start(
                    out=w1all[:, lo:hi, :],
                    in_=w1r_d[128 * lo:128 * hi].rearrange(
                        "(i p) c -> p i c", p=128))

            # conv: cps[(ch,x'), (y,b)] += wband_t.T @ xwin[:, y, t, :]
            # 3 N-slices (one PSUM bank each) x 2 K-tiles, t inner.  Each
            # slice is a y-chunk; relu + repack for the chunk follow
            # immediately, split over the two HWDGE queues (scalar/sync), so
            # the repack overlaps the W1 stream instead of trailing it.
            cps = cpsum_pool.tile([Q, 20, B], F32)
            cps_flat = cps[:].rearrange("q y b -> q (y b)")
            xfT = const.tile([Q, 20, B], BF16)
            xk = xkpool.tile([128, NBLK, B], BF16)
            nc.vector.memset(xk[64:128, NBLK - 1, :], 0)  # pad rows 1600-1663
            rp = 0
            rp_eng = [nc.scalar, nc.gpsimd]
            for ylo, yhi in ((0, 8), (8, 16), (16, 20)):
                for t in range(2):
                    nc.tensor.matmul(
                        cps_flat[:, 64 * ylo:64 * yhi], wb[:, t, :],
                        xw[:, ylo:yhi, t, :],
                        start=(t == 0), stop=(t == 1),
                    )
                # fused bias+relu on DVE (keeps ACT idle -> no table load):
                # xfT = max(cps + bias, 0), cast to bf16
                nc.vector.tensor_scalar(xfT[:, ylo:yhi, :],
                                        cps[:, ylo:yhi, :], bias_t[:], 0.0,
                                        op0=ALU.add, op1=ALU.max)
                # repack this chunk's (q, y) rows into the dense 128-row
                # k-blocks (y-major flat row r = 80y + q), SBUF->SBUF
                for y in range(ylo, yhi):
                    r0 = Q * y
                    i0, off = divmod(r0, 128)
                    seg = min(128 - off, Q)
                    rp_eng[rp % 2].dma_start(out=xk[off:off + seg, i0, :],
                                             in_=xfT[0:seg, y, :])
                    rp += 1
                    if seg < Q:
                        rp_eng[rp % 2].dma_start(out=xk[0:Q - seg, i0 + 1, :],
                                                 in_=xfT[seg:Q, y, :])
                        rp += 1

            # stage 2: for each k-block, 4 matmuls in 2 concurrent col-group
            # pairs (M=64 = batch uses half the PE array; tile_position
            # (0,0)/(0,64) runs two at once).  4 separate PSUM banks so each
            # (h,g) accumulation group owns its bank.
            sps = [spsum_pool.tile([128, 512], F32, tag=f"sp{h}{g}",
                                   name=f"sp{h}{g}")
                   for h in range(2) for g in range(2)]
            for i in range(NBLK):
                for h in range(2):
                    for g in range(2):
                        lo = 1024 * h + 512 * g
                        nc.tensor.matmul(
                            sps[2 * h + g][64 * g:64 * g + 64, :],
                            xk[:, i, :], w1all[:, i, lo:lo + 512],
                            start=(i == 0), stop=(i == NBLK - 1),
                            tile_position=(0, 64 * g),
                        )

            # fold c: PSUM [64, (j2, m, c)] -> s_all[(g,b), (h, j2, m)]
            s_all = const.tile([128, 32], F32)
            for h in range(2):
                for g in range(2):
                    nc.vector.reduce_sum(
                        s_all[64 * g:64 * g + 64, 16 * h:16 * h + 16]
                        .rearrange("b (j m) -> b j m", m=M1),
                        sps[2 * h + g][64 * g:64 * g + 64, :]
                        .rearrange("b (j m c) -> b j m c", m=M1, c=C1),
                        axis=AX.X,
                    )
            nc.sync.dma_start(out=sp_d[:], in_=s_all[:])

    nc.compile()
    return nc


# ----------------------------------------------------------------------------
# phase B: squash -> digit caps -> 3-iter routing  (1 core, tiny tensors)
# ----------------------------------------------------------------------------

def _squash(nc, pool, s_ap, j, m, out_tag, out_dtype=BF16, mj=False,
            pre_scale=1.0):
    """v = |s|/(1+|s|^2) * s with the norm over the m-dim.
    Sqrt is the ONLY ACT function phase B uses (softmax exp is a DVE
    polynomial), so there is exactly one ACT_TABLE_LOAD in the kernel.
    n (ACT) and den/rden (DVE) depend only on ss, so they overlap.
    mj=False: s is [B, (j, m)];  mj=True: s is [B, (m, j)] (routing layout,
    keeps j stride-1 innermost for the big broadcast muls).
    pre_scale=beta computes squash(beta * s) from s without ever scaling s:
    n' = sqrt(beta^4 ss) = beta^2 sqrt(ss) and den = 1 + beta^2 ss fold beta
    into existing op immediates, so v = s * n'/den = squash(beta s)."""
    sq = pool.tile([B, j * m], F32, tag=out_tag + "_sq")
    nc.vector.tensor_mul(sq[:], s_ap, s_ap)
    ss = pool.tile([B, j], F32, tag=out_tag + "_ss")
    if mj:
        ssv = sq[:].rearrange("b (m j) -> b j m", j=j)
    else:
        ssv = sq[:].rearrange("b (j m) -> b j m", m=m)
    nc.vector.reduce_sum(ss[:], ssv, axis=AX.X)
    n = pool.tile([B, j], F32, tag=out_tag + "_n")
    nc.scalar.activation(out=n[:], in_=ss[:], func=AF.Sqrt,
                         scale=pre_scale ** 4)
    den = pool.tile([B, j], F32, tag=out_tag + "_den")
    nc.vector.tensor_scalar(den[:], ss[:], pre_scale ** 2, 1.0,
                            op0=ALU.mult, op1=ALU.add)
    rden = pool.tile([B, j], F32, tag=out_tag + "_rden")
    nc.vector.reciprocal(rden[:], den[:])
    f = pool.tile([B, j], F32, tag=out_tag + "_f")
    nc.vector.tensor_mul(f[:], n[:], rden[:])
    v = pool.tile([B, j * m], out_dtype, tag=out_tag)
    if mj:
        nc.vector.tensor_mul(
            v[:].rearrange("b (m j) -> b m j", j=j),
            s_ap.rearrange("b (m j) -> b m j", j=j),
            f[:].to_broadcast([B, j, m]).rearrange("b j m -> b m j"),
        )
    else:
        nc.vector.tensor_mul(
            v[:].rearrange("b (j m) -> b j m", m=m),
            s_ap.rearrange("b (j m) -> b j m", m=m),
            f[:].to_broadcast([B, j, m]),
        )
    return v


def _build_phase_b():
    nc = bacc.Bacc("TRN2", target_bir_lowering=False, debug=False,
                   num_devices=1)
    s_d = nc.dram_tensor("s", [B, JM], F32, kind="ExternalInput")
    w2bd_d = nc.dram_tensor("w2bd", [K2 * C2, JKM], BF16, kind="ExternalInput")
    ident_d = nc.dram_tensor("ident", [B, B], BF16, kind="ExternalInput")
    v2_d = nc.dram_tensor("v2", [B, J2 * M2], F32, kind="ExternalOutput")

    with tile.TileContext(nc) as tc:
        with (
            tc.tile_pool(name="sb", bufs=1) as sb,
            tc.tile_pool(name="ps", bufs=1, space="PSUM") as ps,
        ):
            s_t = sb.tile([B, JM], F32)
            nc.sync.dma_start(out=s_t[:], in_=s_d[:])
            ident_t = sb.tile([B, B], BF16)
            nc.sync.dma_start(out=ident_t[:], in_=ident_d[:])
            w2bd_t = sb.tile([K2 * C2, JKM], BF16)
            nc.sync.dma_start(out=w2bd_t[:], in_=w2bd_d[:])

            # s is [b, (k, c)] (primary caps j1=k, m1=c), norm over c
            v1 = _squash(nc, sb, s_t[:], J1, M1, "v1")

            # v1T = v1.T via PE so it can be the stationary operand
            tp = ps.tile([B, B], BF16, tag="tp")
            nc.tensor.transpose(tp[:], v1[:], ident_t[:])
            v1t = sb.tile([B, B], BF16)
            nc.vector.tensor_copy(v1t[:], tp[:])

            # u2[b, (k,m,j)] = v1_flat @ w2bd, copied once to SBUF bf16 (DVE
            # ops with a PSUM-f32 operand measured ~1.8x slower than SBUF
            # bf16).  k is outermost so k-reductions are halving adds over
            # contiguous slabs; j is stride-1 innermost so every broadcast
            # operand keeps real data innermost.
            up = ps.tile([B, JKM], F32, tag="up")
            for lo, hi in ((0, 512), (512, 1024), (1024, JKM)):
                nc.tensor.matmul(up[:, lo:hi], v1t[:], w2bd_t[:, lo:hi],
                                 start=True, stop=True)
            u2 = sb.tile([B, JKM], BF16)
            nc.vector.tensor_copy(u2[:], up[:])
            u2_kmj = u2[:].rearrange("b (k m j) -> b k m j", m=M2, j=J2)

            bij = sb.tile([B, K2 * J2], F32)
            tmp = sb.tile([B, K2, M2 * J2], BF16)   # m-tree operand
            tmp2 = sb.tile([B, JKM], BF16)          # k-tree operand
            t4k = sb.tile([B, JKM // 2], BF16)
            t2k = sb.tile([B, JKM // 4], BF16)
            t8m = sb.tile([B, K2, M2 * J2 // 2], BF16)
            t4m = sb.tile([B, K2, M2 * J2 // 4], BF16)
            t2m = sb.tile([B, K2, M2 * J2 // 8], BF16)
            s2 = sb.tile([B, M2 * J2], F32)

            def ktree(src, psum=False):  # sum over k (outermost): halving adds
                if psum:
                    # DVE may read only ONE input from PSUM: copy lo half to
                    # SBUF first, then accumulate the hi half from PSUM
                    nc.vector.tensor_copy(t4k[:], src[:, 0:640])
                    nc.vector.tensor_add(t4k[:], t4k[:], src[:, 640:1280])
                else:
                    nc.vector.tensor_add(t4k[:], src[:, 0:640],
                                         src[:, 640:1280])
                nc.vector.tensor_add(t2k[:], t4k[:, 0:320], t4k[:, 320:640])
                nc.vector.tensor_add(s2[:], t2k[:, 0:160], t2k[:, 160:320])

            def mtree(out_kj):  # sum tmp over m (middle): 4 halving adds
                nc.vector.tensor_add(t8m[:], tmp[:, :, 0:80], tmp[:, :, 80:160])
                nc.vector.tensor_add(t4m[:], t8m[:, :, 0:40], t8m[:, :, 40:80])
                nc.vector.tensor_add(t2m[:], t4m[:, :, 0:20], t4m[:, :, 20:40])
                nc.vector.tensor_add(out_kj, t2m[:, :, 0:10], t2m[:, :, 10:20])

            v = None
            for it in range(3):
                if it == 0:
                    # softmax of zeros over j is uniform: s2 = 0.1 * sum_k u2
                    ktree(u2)
                else:
                    # routing logits are ~1e-3, so exp(b) = 1 + b(1 + b/2) to
                    # fp32 accuracy (softmax only needs the ratios); this
                    # keeps exp off ACT so Sqrt never alternates table sets
                    eu = sb.tile([B, K2 * J2], F32, tag="eu")
                    nc.vector.tensor_scalar(eu[:], bij[:], 0.5, 1.0,
                                            op0=ALU.mult, op1=ALU.add)
                    e = sb.tile([B, K2 * J2], F32, tag="e")
                    nc.vector.tensor_mul(e[:], bij[:], eu[:])
                    nc.vector.tensor_scalar_add(e[:], e[:], 1.0)
                    dn = sb.tile([B, K2], F32, tag="dn")
                    nc.vector.reduce_sum(
                        dn[:], e[:].rearrange("b (k j) -> b k j", j=J2),
                        axis=AX.X)
                    rdn = sb.tile([B, K2], F32, tag="rdn")
                    nc.vector.reciprocal(rdn[:], dn[:])
                    c = sb.tile([B, K2 * J2], BF16, tag="c")
                    nc.vector.tensor_mul(
                        c[:].rearrange("b (k j) -> b k j", j=J2),
                        e[:].rearrange("b (k j) -> b k j", j=J2),
                        rdn[:].to_broadcast([B, K2, J2]),
                    )
                    # s2[b,m,j] = sum_k c[b,k,j] * u2[b,k,m,j]
                    nc.vector.tensor_mul(
                        tmp2[:].rearrange("b (k m j) -> b k m j",
                                          m=M2, j=J2),
                        u2_kmj,
                        c[:].rearrange("b (k j) -> b k j", j=J2)
                        .to_broadcast([B, K2, J2, M2])
                        .rearrange("b k j m -> b k m j"),
                    )
                    ktree(tmp2)
                v = _squash(nc, sb, s2[:], J2, M2, "v", mj=True,
                            out_dtype=(F32 if it == 2 else BF16),
                            pre_scale=(1.0 / J2 if it == 0 else 1.0))
                if it < 2:
                    # bij[b,k,j] += sum_m u2[b,k,m,j] * v[b,m,j]
                    nc.vector.tensor_mul(
                        tmp[:].rearrange("b k (m j) -> b k m j", j=J2),
                        u2_kmj,
                        v[:].rearrange("b (m j) -> b m j", j=J2)
                        .to_broadcast([B, M2, J2, K2])
                        .rearrange("b m j k -> b k m j"),
                    )
                    if it == 0:
                        mtree(bij[:].rearrange("b (k j) -> b k j", j=J2))
                    else:
                        bupd = sb.tile([B, K2 * J2], F32, tag="bupd")
                        mtree(bupd[:].rearrange("b (k j) -> b k j", j=J2))
                        nc.vector.tensor_add(bij[:], bij[:], bupd[:])

            # v is [b, (m, j)]; the host transposes to [b, (j, m)]
            nc.sync.dma_start(out=v2_d[:], in_=v[:])

    nc.compile()
    return nc


# ----------------------------------------------------------------------------
# entry point
# ----------------------------------------------------------------------------

LAST_RESULTS = []  # [phase_a BassKernelResults, phase_b BassKernelResults]


def kernel(x, conv_w, conv_b, W1, W2):
    x = np.ascontiguousarray(np.asarray(x, np.float32))
    conv_w = np.asarray(conv_w, np.float32)
    conv_b = np.asarray(conv_b, np.float32)
    W1 = np.asarray(W1, np.float32)
    W2 = np.asarray(W2, np.float32)

    if "a" not in _CACHE:
        _CACHE["a"] = _build_phase_a()
        _CACHE["b"] = _build_phase_b()
    nca, ncb = _CACHE["a"], _CACHE["b"]

    xwin = _prep_xwin(x)
    in_maps = []
    for i in range(NCORES):
        in_maps.append({
            "xwin": xwin,
            "wband": _prep_wband(conv_w, NCH * i),
            "bias": np.ascontiguousarray(
                np.repeat(conv_b[NCH * i:NCH * i + NCH] * 0.125, 20)
            ).reshape(Q, 1),
            "w1r": _prep_w1r(W1, NCH * i),
        })

    ra = run_bass_kernel_spmd(nca, in_maps, list(range(NCORES)))
    s128 = np.sum([r["sp"] for r in ra.results], axis=0, dtype=np.float32)
    # [(g,b), (h,j2,m)] -> [b, (j=4h+2g+j2, m)]
    s = np.ascontiguousarray(
        s128.reshape(2, B, 2, 2, M1).transpose(1, 2, 0, 3, 4).reshape(B, JM))

    rb = run_bass_kernel_spmd(
        ncb,
        [{"s": s, "w2bd": _prep_w2bd(W2),
          "ident": np.eye(B, dtype=NPBF)}],
        [0],
    )
    LAST_RESULTS[:] = [ra, rb]
    # v2 comes back in (m, j) order; transpose to [b, j, m]
    return np.ascontiguousarray(
        rb.results[0]["v2"].reshape(B, M2, J2).transpose(0, 2, 1))


# revision 28
# speedup vs baseline: 1.0714x; 1.0714x over previous
"""CapsuleNetwork forward on 8 Trainium2 NeuronCores (Bass/Tile).

Math (validated against the jax reference in a numpy prototype):
  conv+relu:  h = relu(conv2d(x, conv_w) + conv_b)            [64,32,20,20]
  stage 2:    u1 = einsum('jkmc,bk->bjkm', W1, h.flat)  and routing(u1, 1)
              collapses (softmax of zeros is uniform 1/8) to
                s[b,j,m] = (1/8) * sum_k h.flat[b,k] * sum_c W1[j,k,m,c]
              i.e. ONE matmul contracting k, with the c-reduction done in
              PSUM columns and folded by a vector reduce at the end.
  v1 = squash(s);  u2 = einsum('jkmc,bkc->bjkm', W2, v1);  v2 = routing(u2, 3)

Sharding: the W1 stream dominates (memory regime), so we shard the
contraction k = (ch, y, x') by conv CHANNEL: core i owns channels 4i..4i+3,
computes its 4-channel slice of the conv and the partial s over its 1600 k's
(every element of W1 read exactly once chip-wide).  W1 is streamed in bf16
(weight-only quantization; whole-pipeline rel err ~5e-3 vs the 2e-2 gate),
which halves the HBM traffic that bounds this kernel.  Partials [128,32] are
gathered and summed on host (the unshard step), then a small phase-B kernel
on core 0 runs squash -> digit-caps matmul -> 3-iter routing.  (A fused
single-kernel variant with an on-device AllReduce was measured: the 8-core
launch skew under this runner is ~60us, so any cross-core sync inside the
kernel inflates the measured span.  Two independent launches win.)

The conv is expressed as 2 stationary banded-weight matmuls so its output
lands directly in the [k-on-partitions, batch] layout stage 2 needs.
Host prep is layout only (transpose/slice/band-expansion/dtype of weights).
"""

import contextlib
import ctypes
import os
import sys
import types

os.environ.setdefault("NEURON_RT_RESET_CORES", "1")  # recover wedged cores


def _install_axon_ntff_shim():
    """concourse.bass_utils imports antenv.axon_hooks for trace=True under
    axon; this image's antenv lacks that module. Recreate the documented
    ctypes hook (see trn_agent_boot) so tracing works instead of crashing."""
    try:
        import antenv.axon_hooks  # noqa: F401
        return
    except ImportError:
        pass

    def _make_hook():
        so_path = "/opt/axon/libaxon_pjrt.so"
        if not os.path.exists(so_path):
            return None
        lib = ctypes.CDLL(so_path)
        if not hasattr(lib, "axon_start_nrt_profile"):
            return None
        lib.axon_start_nrt_profile.argtypes = [
            ctypes.POINTER(ctypes.c_int64), ctypes.c_size_t]
        lib.axon_start_nrt_profile.restype = ctypes.c_int64
        lib.axon_stop_nrt_profile.argtypes = [ctypes.c_char_p]
        lib.axon_stop_nrt_profile.restype = ctypes.c_int64

        @contextlib.contextmanager
        def _hook(output_dir, device_ids):
            import jax
            jax.devices()
            if device_ids:
                ids = (ctypes.c_int64 * len(device_ids))(*device_ids)
                rc = lib.axon_start_nrt_profile(ids, len(device_ids))
            else:
                rc = lib.axon_start_nrt_profile(None, 0)
            if rc != 0:
                raise RuntimeError(f"axon_start_nrt_profile rc={rc}")
            try:
                yield
            finally:
                n = lib.axon_stop_nrt_profile(str(output_dir).encode())
                print(f"profile: {n} file(s) written to {output_dir}",
                      file=sys.stderr)

        return _hook

    mod = types.ModuleType("antenv.axon_hooks")
    hook = _make_hook()
    mod.get_axon_ntff_profile_hook = lambda: hook
    mod.set_axon_ntff_profile_hook = lambda h: None
    sys.modules["antenv.axon_hooks"] = mod


_install_axon_ntff_shim()

import ml_dtypes
import numpy as np

import concourse.bacc as bacc
import concourse.bass as bass
import concourse.tile as tile
from concourse import mybir
from concourse.bass_utils import run_bass_kernel_spmd

F32 = mybir.dt.float32
BF16 = mybir.dt.bfloat16
NPBF = ml_dtypes.bfloat16
AX = mybir.AxisListType
AF = mybir.ActivationFunctionType
ALU = mybir.AluOpType

B = 64          # batch
NCORES = 8
NCH = 4         # conv channels per core
P1 = 126        # conv contraction tile (2 tiles cover the 9x28 input window)
Q = NCH * 20    # 80 = (ch, x') partitions per core
J1, M1, C1 = 8, 8, 32
J2, K2, M2, C2 = 10, 8, 16, 8
JM = J1 * M1    # 64
JKM = J2 * K2 * M2  # 1280
NROW = 20 * Q   # 1600 k-rows per core
NBLK = 13       # ceil(1600 / 128) stationary k-blocks

_CACHE = {}

# ----------------------------------------------------------------------------
# host-side relayout helpers (layout + dtype only, no model arithmetic)
# ----------------------------------------------------------------------------

def _prep_xwin(x):
    """xwin[p, y, t, b] = xT[28y + 126t + p, b]: the two 126-row K-tiles of
    the 9-row input window for each conv output row y, partition-major and
    y-second so each conv y-chunk loads as one contiguous DMA slice."""
    xT = np.ascontiguousarray(x.reshape(B, 784).T)            # [pix, b]
    p = np.arange(P1)[:, None, None]
    y = np.arange(20)[None, :, None]
    t = np.arange(2)[None, None, :]
    rows = 28 * y + P1 * t + p                                # [126,20,2]
    return np.ascontiguousarray(xT[rows].astype(NPBF))        # [126,20,2,64]


def _prep_wband(conv_w, ch_lo):
    """wband[p, t, (ch,x')] = conv_w[ch_lo+ch, 0, dy, xin-x'] / 8
    where (dy, xin) = divmod(126t + p, 28).  The 1/8 is the uniform
    softmax coupling of routing(u1, 1), folded into the (linear) conv;
    relu(z/8) == relu(z)/8."""
    wb = np.zeros((252, NCH, 20), np.float32)
    cw = conv_w[ch_lo:ch_lo + NCH, 0]                         # [4, 9, 9]
    for idx in range(252):
        dy, xin = divmod(idx, 28)
        for xp in range(max(0, xin - 8), min(20, xin + 1)):
            wb[idx, :, xp] = cw[:, dy, xin - xp] * 0.125
    return np.ascontiguousarray(
        wb.reshape(2, P1, Q).transpose(1, 0, 2).astype(NPBF))  # [126,2,80]


def _prep_w1r(W1, ch_lo):
    """w1r[(y, ch, x'), (j, m, c)] — this core's k-rows of W1 in bf16,
    zero-padded 1600 -> 1664 rows so the stream is 13 full 128-partition
    tiles (zero rows multiply against garbage xk pad rows, contributing 0)."""
    v = W1.reshape(J1, 32, 20, 20, M1 * C1)[:, ch_lo:ch_lo + NCH]
    v = v.transpose(2, 1, 3, 0, 4).reshape(NROW, J1 * M1 * C1)
    out = np.zeros((NBLK * 128, J1 * M1 * C1), NPBF)
    out[:NROW] = v.astype(NPBF)
    return out


def _prep_w2bd(W2):
    """Block-diagonal digit-caps weights, output columns ordered (k, m, j):
    bd[(k',c), (k,m,j)] = W2[j,k,m,c] iff k == k', so u2 = v1_flat @ bd in
    one matmul.  The (k,m,j) order makes every big routing op walk with a
    real stride-1 innermost dim: k-reductions and m-reductions become
    halving adds over contiguous slabs, and the c/v broadcasts put their
    0-stride dim outermost (0-stride innermost measured 1.8x slower)."""
    bd5 = np.zeros((K2, C2, K2, M2, J2), np.float32)
    for k in range(K2):
        bd5[k, :, k, :, :] = W2[:, k, :, :].transpose(2, 1, 0)
    return np.ascontiguousarray(bd5.reshape(K2 * C2, JKM).astype(NPBF))


# ----------------------------------------------------------------------------
# phase A: per-core conv + partial capsule matmul  (SPMD on 8 cores)
# ----------------------------------------------------------------------------

def _build_phase_a():
    nc = bacc.Bacc("TRN2", target_bir_lowering=False, debug=False,
                   num_devices=NCORES)
    xwin_d = nc.dram_tensor("xwin", [P1, 20, 2, B], BF16, kind="ExternalInput")
    wband_d = nc.dram_tensor("wband", [P1, 2, Q], BF16, kind="ExternalInput")
    bias_d = nc.dram_tensor("bias", [Q, 1], F32, kind="ExternalInput")
    w1r_d = nc.dram_tensor("w1r", [NBLK * 128, J1 * M1 * C1], BF16,
                           kind="ExternalInput")
    sp_d = nc.dram_tensor("sp", [128, 32], F32, kind="ExternalOutput")

    with tile.TileContext(nc) as tc:
        with (
            tc.tile_pool(name="const", bufs=1) as const,
            tc.tile_pool(name="w1pool", bufs=1) as w1pool,
            tc.tile_pool(name="xkpool", bufs=1) as xkpool,
            tc.tile_pool(name="cpsum", bufs=1, space="PSUM") as cpsum_pool,
            tc.tile_pool(name="spsum", bufs=1, space="PSUM") as spsum_pool,
        ):
            # conv inputs first on sync; xw in 3 y-chunk DMAs so conv chunk 1
            # starts as soon as its slice lands
            xw = const.tile([P1, 20, 2, B], BF16)
            nc.sync.dma_start(out=xw[:, 0:8, :, :], in_=xwin_d[:, 0:8, :, :])
            wb = const.tile([P1, 2, Q], BF16)
            nc.sync.dma_start(out=wb[:], in_=wband_d[:])
            bias_t = const.tile([Q, 1], F32)
            nc.sync.dma_start(out=bias_t[:], in_=bias_d[:])
            nc.sync.dma_start(out=xw[:, 8:16, :, :], in_=xwin_d[:, 8:16, :, :])
            nc.sync.dma_start(out=xw[:, 16:20, :, :],
                              in_=xwin_d[:, 16:20, :, :])

            # W1 stream: one [128, 13*4KB] tile fed by 4 big DMAs (1.6-2 MB
            # each -- small transfers sit below the SDMA efficiency knee) on
            # gpsimd (SWDGE sustains ~350+ GB/s solo; HWDGE measured ~250).
            # Concurrent queues do NOT share SDMA bandwidth fairly (a
            # megabyte-transfer queue starves a small one ~8:1, measured), so
            # each W1 chunk is write-after-write gated behind a tiny vector
            # copy that depends on xw chunk 2: the W1 stream starts only
            # after the conv input is resident.  Total HBM bytes are
            # unchanged, so the stream still ends at the same time, but conv
            # + repack start ~7 us earlier.  (Program order alone cannot do
            # this: the Tile scheduler reorders queue ops by readiness.)
            w1all = w1pool.tile([128, NBLK, J1 * M1 * C1], BF16)
            gate_src = xw[:, 8, :, :].rearrange("p t b -> p (t b)")
            for lo, hi in ((0, 4), (4, 8), (8, 12), (12, 13)):
                nc.vector.tensor_copy(w1all[0:P1, lo, 0:2 * B], gate_src)
                nc.gpsimd.dma_start(
                    out=w1all[:, lo:hi, :],
                    in_=w1r_d[128 * lo:128 * hi].rearrange(
                        "(i p) c -> p i c", p=128))

            # conv: cps[(ch,x'), (y,b)] += wband_t.T @ xwin[:, y, t, :]
            # 3 N-slices (one PSUM bank each) x 2 K-tiles, t inner.  Each
            # slice is a y-chunk; relu + repack for the chunk follow
            # immediately, split over the two HWDGE queues (scalar/sync), so
            # the repack overlaps the W1 stream instead of trailing it.
            cps = cpsum_pool.tile([Q, 20, B], F32)
            cps_flat = cps[:].rearrange("q y b -> q (y b)")
            xfT = const.tile([Q, 20, B], BF16)
            xk = xkpool.tile([128, NBLK, B], BF16)
            nc.vector.memset(xk[64:128, NBLK - 1, :], 0)  # pad rows 1600-1663
            rp = 0
            rp_eng = [nc.scalar, nc.sync]
            for ylo, yhi in ((0, 8), (8, 16), (16, 20)):
                for t in range(2):
                    nc.tensor.matmul(
                        cps_flat[:, 64 * ylo:64 * yhi], wb[:, t, :],
                        xw[:, ylo:yhi, t, :],
                        start=(t == 0), stop=(t == 1),
                    )
                # fused bias+relu on DVE (keeps ACT idle -> no table load):
                # xfT = max(cps + bias, 0), cast to bf16
                nc.vector.tensor_scalar(xfT[:, ylo:yhi, :],
                                        cps[:, ylo:yhi, :], bias_t[:], 0.0,
                                        op0=ALU.add, op1=ALU.max)
                # repack this chunk's (q, y) rows into the dense 128-row
                # k-blocks (y-major flat row r = 80y + q), SBUF->SBUF
                for y in range(ylo, yhi):
                    r0 = Q * y
                    i0, off = divmod(r0, 128)
                    seg = min(128 - off, Q)
                    rp_eng[rp % 2].dma_start(out=xk[off:off + seg, i0, :],
                                             in_=xfT[0:seg, y, :])
                    rp += 1
                    if seg < Q:
                        rp_eng[rp % 2].dma_start(out=xk[0:Q - seg, i0 + 1, :],
                                                 in_=xfT[seg:Q, y, :])
                        rp += 1

            # stage 2: for each k-block, 4 matmuls in 2 concurrent col-group
            # pairs (M=64 = batch uses half the PE array; tile_position
            # (0,0)/(0,64) runs two at once).  4 separate PSUM banks so each
            # (h,g) accumulation group owns its bank.
            sps = [spsum_pool.tile([128, 512], F32, tag=f"sp{h}{g}",
                                   name=f"sp{h}{g}")
                   for h in range(2) for g in range(2)]
            for i in range(NBLK):
                for h in range(2):
                    for g in range(2):
                        lo = 1024 * h + 512 * g
                        nc.tensor.matmul(
                            sps[2 * h + g][64 * g:64 * g + 64, :],
                            xk[:, i, :], w1all[:, i, lo:lo + 512],
                            start=(i == 0), stop=(i == NBLK - 1),
                            tile_position=(0, 64 * g),
                        )

            # fold c: PSUM [64, (j2, m, c)] -> s_all[(g,b), (h, j2, m)]
            s_all = const.tile([128, 32], F32)
            for h in range(2):
                for g in range(2):
                    nc.vector.reduce_sum(
                        s_all[64 * g:64 * g + 64, 16 * h:16 * h + 16]
                        .rearrange("b (j m) -> b j m", m=M1),
                        sps[2 * h + g][64 * g:64 * g + 64, :]
                        .rearrange("b (j m c) -> b j m c", m=M1, c=C1),
                        axis=AX.X,
                    )
            nc.sync.dma_start(out=sp_d[:], in_=s_all[:])

    nc.compile()
    return nc


# ----------------------------------------------------------------------------
# phase B: squash -> digit caps -> 3-iter routing  (1 core, tiny tensors)
# ----------------------------------------------------------------------------

def _squash(nc, pool, s_ap, j, m, out_tag, out_dtype=BF16, mj=False,
            pre_scale=1.0):
    """v = |s|/(1+|s|^2) * s with the norm over the m-dim.
    Sqrt is the ONLY ACT function phase B uses (softmax exp is a DVE
    polynomial), so there is exactly one ACT_TABLE_LOAD in the kernel.
    n (ACT) and den/rden (DVE) depend only on ss, so they overlap.
    mj=False: s is [B, (j, m)];  mj=True: s is [B, (m, j)] (routing layout,
    keeps j stride-1 innermost for the big broadcast muls).
    pre_scale=beta computes squash(beta * s) from s without ever scaling s:
    n' = sqrt(beta^4 ss) = beta^2 sqrt(ss) and den = 1 + beta^2 ss fold beta
    into existing op immediates, so v = s * n'/den = squash(beta s)."""
    sq = pool.tile([B, j * m], F32, tag=out_tag + "_sq")
    nc.vector.tensor_mul(sq[:], s_ap, s_ap)
    ss = pool.tile([B, j], F32, tag=out_tag + "_ss")
    if mj:
        ssv = sq[:].rearrange("b (m j) -> b j m", j=j)
    else:
        ssv = sq[:].rearrange("b (j m) -> b j m", m=m)
    nc.vector.reduce_sum(ss[:], ssv, axis=AX.X)
    n = pool.tile([B, j], F32, tag=out_tag + "_n")
    nc.scalar.activation(out=n[:], in_=ss[:], func=AF.Sqrt,
                         scale=pre_scale ** 4)
    den = pool.tile([B, j], F32, tag=out_tag + "_den")
    nc.vector.tensor_scalar(den[:], ss[:], pre_scale ** 2, 1.0,
                            op0=ALU.mult, op1=ALU.add)
    rden = pool.tile([B, j], F32, tag=out_tag + "_rden")
    nc.vector.reciprocal(rden[:], den[:])
    f = pool.tile([B, j], F32, tag=out_tag + "_f")
    nc.vector.tensor_mul(f[:], n[:], rden[:])
    v = pool.tile([B, j * m], out_dtype, tag=out_tag)
    if mj:
        nc.vector.tensor_mul(
            v[:].rearrange("b (m j) -> b m j", j=j),
            s_ap.rearrange("b (m j) -> b m j", j=j),
            f[:].to_broadcast([B, j, m]).rearrange("b j m -> b m j"),
        )
    else:
        nc.vector.tensor_mul(
            v[:].rearrange("b (j m) -> b j m", m=m),
            s_ap.rearrange("b (j m) -> b j m", m=m),
            f[:].to_broadcast([B, j, m]),
        )
    return v


def _build_phase_b():
    nc = bacc.Bacc("TRN2", target_bir_lowering=False, debug=False,
                   num_devices=1)
    s_d = nc.dram_tensor("s", [B, JM], F32, kind="ExternalInput")
    w2bd_d = nc.dram_tensor("w2bd", [K2 * C2, JKM], BF16, kind="ExternalInput")
    ident_d = nc.dram_tensor("ident", [B, B], BF16, kind="ExternalInput")
    v2_d = nc.dram_tensor("v2", [B, J2 * M2], F32, kind="ExternalOutput")

    with tile.TileContext(nc) as tc:
        with (
            tc.tile_pool(name="sb", bufs=1) as sb,
            tc.tile_pool(name="ps", bufs=1, space="PSUM") as ps,
        ):
            s_t = sb.tile([B, JM], F32)
            nc.sync.dma_start(out=s_t[:], in_=s_d[:])
            ident_t = sb.tile([B, B], BF16)
            nc.sync.dma_start(out=ident_t[:], in_=ident_d[:])
            w2bd_t = sb.tile([K2 * C2, JKM], BF16)
            nc.sync.dma_start(out=w2bd_t[:], in_=w2bd_d[:])

            # s is [b, (k, c)] (primary caps j1=k, m1=c), norm over c
            v1 = _squash(nc, sb, s_t[:], J1, M1, "v1")

            # v1T = v1.T via PE so it can be the stationary operand
            tp = ps.tile([B, B], BF16, tag="tp")
            nc.tensor.transpose(tp[:], v1[:], ident_t[:])
            v1t = sb.tile([B, B], BF16)
            nc.vector.tensor_copy(v1t[:], tp[:])

            # u2[b, (k,m,j)] = v1_flat @ w2bd, copied once to SBUF bf16 (DVE
            # ops with a PSUM-f32 operand measured ~1.8x slower than SBUF
            # bf16).  k is outermost so k-reductions are halving adds over
            # contiguous slabs; j is stride-1 innermost so every broadcast
            # operand keeps real data innermost.
            up = ps.tile([B, JKM], F32, tag="up")
            for lo, hi in ((0, 512), (512, 1024), (1024, JKM)):
                nc.tensor.matmul(up[:, lo:hi], v1t[:], w2bd_t[:, lo:hi],
                                 start=True, stop=True)
            u2 = sb.tile([B, JKM], BF16)
            nc.vector.tensor_copy(u2[:], up[:])
            u2_kmj = u2[:].rearrange("b (k m j) -> b k m j", m=M2, j=J2)

            bij = sb.tile([B, K2 * J2], F32)
            tmp = sb.tile([B, K2, M2 * J2], BF16)   # m-tree operand
            tmp2 = sb.tile([B, JKM], BF16)          # k-tree operand
            t4k = sb.tile([B, JKM // 2], BF16)
            t2k = sb.tile([B, JKM // 4], BF16)
            t8m = sb.tile([B, K2, M2 * J2 // 2], BF16)
            t4m = sb.tile([B, K2, M2 * J2 // 4], BF16)
            t2m = sb.tile([B, K2, M2 * J2 // 8], BF16)
            s2 = sb.tile([B, M2 * J2], F32)

            def ktree(src, psum=False):  # sum over k (outermost): halving adds
                if psum:
                    # DVE may read only ONE input from PSUM: copy lo half to
                    # SBUF first, then accumulate the hi half from PSUM
                    nc.vector.tensor_copy(t4k[:], src[:, 0:640])
                    nc.vector.tensor_add(t4k[:], t4k[:], src[:, 640:1280])
                else:
                    nc.vector.tensor_add(t4k[:], src[:, 0:640],
                                         src[:, 640:1280])
                nc.vector.tensor_add(t2k[:], t4k[:, 0:320], t4k[:, 320:640])
                nc.vector.tensor_add(s2[:], t2k[:, 0:160], t2k[:, 160:320])

            def mtree(out_kj):  # sum tmp over m (middle): 4 halving adds
                nc.vector.tensor_add(t8m[:], tmp[:, :, 0:80], tmp[:, :, 80:160])
                nc.vector.tensor_add(t4m[:], t8m[:, :, 0:40], t8m[:, :, 40:80])
                nc.vector.tensor_add(t2m[:], t4m[:, :, 0:20], t4m[:, :, 20:40])
                nc.vector.tensor_add(out_kj, t2m[:, :, 0:10], t2m[:, :, 10:20])

            v = None
            for it in range(3):
                if it == 0:
                    # softmax of zeros over j is uniform: s2 = 0.1 * sum_k u2
                    ktree(u2)
                else:
                    # routing logits are ~1e-3, so exp(b) = 1 + b(1 + b/2) to
                    # fp32 accuracy (softmax only needs the ratios); this
                    # keeps exp off ACT so Sqrt never alternates table sets
                    eu = sb.tile([B, K2 * J2], F32, tag="eu")
                    nc.vector.tensor_scalar(eu[:], bij[:], 0.5, 1.0,
                                            op0=ALU.mult, op1=ALU.add)
                    e = sb.tile([B, K2 * J2], F32, tag="e")
                    nc.vector.tensor_mul(e[:], bij[:], eu[:])
                    nc.vector.tensor_scalar_add(e[:], e[:], 1.0)
                    dn = sb.tile([B, K2], F32, tag="dn")
                    nc.vector.reduce_sum(
                        dn[:], e[:].rearrange("b (k j) -> b k j", j=J2),
                        axis=AX.X)
                    rdn = sb.tile([B, K2], F32, tag="rdn")
                    nc.vector.reciprocal(rdn[:], dn[:])
                    c = sb.tile([B, K2 * J2], BF16, tag="c")
                    nc.vector.tensor_mul(
                        c[:].rearrange("b (k j) -> b k j", j=J2),
                        e[:].rearrange("b (k j) -> b k j", j=J2),
                        rdn[:].to_broadcast([B, K2, J2]),
                    )
                    # s2[b,m,j] = sum_k c[b,k,j] * u2[b,k,m,j]
                    nc.vector.tensor_mul(
                        tmp2[:].rearrange("b (k m j) -> b k m j",
                                          m=M2, j=J2),
                        u2_kmj,
                        c[:].rearrange("b (k j) -> b k j", j=J2)
                        .to_broadcast([B, K2, J2, M2])
                        .rearrange("b k j m -> b k m j"),
                    )
                    ktree(tmp2)
                v = _squash(nc, sb, s2[:], J2, M2, "v", mj=True,
                            out_dtype=(F32 if it == 2 else BF16),
                            pre_scale=(1.0 / J2 if it == 0 else 1.0))
                if it < 2:
                    # bij[b,k,j] += sum_m u2[b,k,m,j] * v[b,m,j]
                    nc.vector.tensor_mul(
                        tmp[:].rearrange("b k (m j) -> b k m j", j=J2),
                        u2_kmj,
                        v[:].rearrange("b (m j) -> b m j", j=J2)
                        .to_broadcast([B, M2, J2, K2])
                        .rearrange("b m j k -> b k m j"),
                    )
                    if it == 0:
                        mtree(bij[:].rearrange("b (k j) -> b k j", j=J2))
                    else:
                        bupd = sb.tile([B, K2 * J2], F32, tag="bupd")
                        mtree(bupd[:].rearrange("b (k j) -> b k j", j=J2))
                        nc.vector.tensor_add(bij[:], bij[:], bupd[:])

            # v is [b, (m, j)]; the host transposes to [b, (j, m)]
            nc.sync.dma_start(out=v2_d[:], in_=v[:])

    nc.compile()
    return nc


# ----------------------------------------------------------------------------
# entry point
# ----------------------------------------------------------------------------

LAST_RESULTS = []  # [phase_a BassKernelResults, phase_b BassKernelResults]


def kernel(x, conv_w, conv_b, W1, W2):
    x = np.ascontiguousarray(np.asarray(x, np.float32))
    conv_w = np.asarray(conv_w, np.float32)
    conv_b = np.asarray(conv_b, np.float32)
    W1 = np.asarray(W1, np.float32)
    W2 = np.asarray(W2, np.float32)

    if "a" not in _CACHE:
        _CACHE["a"] = _build_phase_a()
        _CACHE["b"] = _build_phase_b()
    nca, ncb = _CACHE["a"], _CACHE["b"]

    xwin = _prep_xwin(x)
    in_maps = []
    for i in range(NCORES):
        in_maps.append({
            "xwin": xwin,
            "wband": _prep_wband(conv_w, NCH * i),
            "bias": np.ascontiguousarray(
                np.repeat(conv_b[NCH * i:NCH * i + NCH] * 0.125, 20)
            ).reshape(Q, 1),
            "w1r": _prep_w1r(W1, NCH * i),
        })

    ra = run_bass_kernel_spmd(nca, in_maps, list(range(NCORES)))
    s128 = np.sum([r["sp"] for r in ra.results], axis=0, dtype=np.float32)
    # [(g,b), (h,j2,m)] -> [b, (j=4h+2g+j2, m)]
    s = np.ascontiguousarray(
        s128.reshape(2, B, 2, 2, M1).transpose(1, 2, 0, 3, 4).reshape(B, JM))

    rb = run_bass_kernel_spmd(
        ncb,
        [{"s": s, "w2bd": _prep_w2bd(W2),
          "ident": np.eye(B, dtype=NPBF)}],
        [0],
    )
    LAST_RESULTS[:] = [ra, rb]
    # v2 comes back in (m, j) order; transpose to [b, j, m]
    return np.ascontiguousarray(
        rb.results[0]["v2"].reshape(B, M2, J2).transpose(0, 2, 1))


# revision 31
# speedup vs baseline: 1.0881x; 1.0156x over previous
"""CapsuleNetwork forward on 8 Trainium2 NeuronCores (Bass/Tile).

Math (validated against the jax reference in a numpy prototype):
  conv+relu:  h = relu(conv2d(x, conv_w) + conv_b)            [64,32,20,20]
  stage 2:    u1 = einsum('jkmc,bk->bjkm', W1, h.flat)  and routing(u1, 1)
              collapses (softmax of zeros is uniform 1/8) to
                s[b,j,m] = (1/8) * sum_k h.flat[b,k] * sum_c W1[j,k,m,c]
              i.e. ONE matmul contracting k, with the c-reduction done in
              PSUM columns and folded by a vector reduce at the end.
  v1 = squash(s);  u2 = einsum('jkmc,bkc->bjkm', W2, v1);  v2 = routing(u2, 3)

Sharding: the W1 stream dominates (memory regime), so we shard the
contraction k = (ch, y, x') by conv CHANNEL: core i owns channels 4i..4i+3,
computes its 4-channel slice of the conv and the partial s over its 1600 k's
(every element of W1 read exactly once chip-wide).  W1 is streamed in bf16
(weight-only quantization; whole-pipeline rel err ~5e-3 vs the 2e-2 gate),
which halves the HBM traffic that bounds this kernel.  Partials [128,32] are
gathered and summed on host (the unshard step), then a small phase-B kernel
on core 0 runs squash -> digit-caps matmul -> 3-iter routing.  (A fused
single-kernel variant with an on-device AllReduce was measured: the 8-core
launch skew under this runner is ~60us, so any cross-core sync inside the
kernel inflates the measured span.  Two independent launches win.)

The conv is expressed as 2 stationary banded-weight matmuls so its output
lands directly in the [k-on-partitions, batch] layout stage 2 needs.
Host prep is layout only (transpose/slice/band-expansion/dtype of weights).
"""

import contextlib
import ctypes
import os
import sys
import types

os.environ.setdefault("NEURON_RT_RESET_CORES", "1")  # recover wedged cores


def _install_axon_ntff_shim():
    """concourse.bass_utils imports antenv.axon_hooks for trace=True under
    axon; this image's antenv lacks that module. Recreate the documented
    ctypes hook (see trn_agent_boot) so tracing works instead of crashing."""
    try:
        import antenv.axon_hooks  # noqa: F401
        return
    except ImportError:
        pass

    def _make_hook():
        so_path = "/opt/axon/libaxon_pjrt.so"
        if not os.path.exists(so_path):
            return None
        lib = ctypes.CDLL(so_path)
        if not hasattr(lib, "axon_start_nrt_profile"):
            return None
        lib.axon_start_nrt_profile.argtypes = [
            ctypes.POINTER(ctypes.c_int64), ctypes.c_size_t]
        lib.axon_start_nrt_profile.restype = ctypes.c_int64
        lib.axon_stop_nrt_profile.argtypes = [ctypes.c_char_p]
        lib.axon_stop_nrt_profile.restype = ctypes.c_int64

        @contextlib.contextmanager
        def _hook(output_dir, device_ids):
            import jax
            jax.devices()
            if device_ids:
                ids = (ctypes.c_int64 * len(device_ids))(*device_ids)
                rc = lib.axon_start_nrt_profile(ids, len(device_ids))
            else:
                rc = lib.axon_start_nrt_profile(None, 0)
            if rc != 0:
                raise RuntimeError(f"axon_start_nrt_profile rc={rc}")
            try:
                yield
            finally:
                n = lib.axon_stop_nrt_profile(str(output_dir).encode())
                print(f"profile: {n} file(s) written to {output_dir}",
                      file=sys.stderr)

        return _hook

    mod = types.ModuleType("antenv.axon_hooks")
    hook = _make_hook()
    mod.get_axon_ntff_profile_hook = lambda: hook
    mod.set_axon_ntff_profile_hook = lambda h: None
    sys.modules["antenv.axon_hooks"] = mod


_install_axon_ntff_shim()

import ml_dtypes
import numpy as np

import concourse.bacc as bacc
import concourse.bass as bass
import concourse.tile as tile
from concourse import mybir
from concourse.bass_utils import run_bass_kernel_spmd

F32 = mybir.dt.float32
BF16 = mybir.dt.bfloat16
NPBF = ml_dtypes.bfloat16
AX = mybir.AxisListType
AF = mybir.ActivationFunctionType
ALU = mybir.AluOpType

B = 64          # batch
NCORES = 8
NCH = 4         # conv channels per core
P1 = 126        # conv contraction tile (2 tiles cover the 9x28 input window)
Q = NCH * 20    # 80 = (ch, x') partitions per core
J1, M1, C1 = 8, 8, 32
J2, K2, M2, C2 = 10, 8, 16, 8
JM = J1 * M1    # 64
JKM = J2 * K2 * M2  # 1280
NROW = 20 * Q   # 1600 k-rows per core
NBLK = 13       # ceil(1600 / 128) stationary k-blocks

_CACHE = {}

# ----------------------------------------------------------------------------
# host-side relayout helpers (layout + dtype only, no model arithmetic)
# ----------------------------------------------------------------------------

def _prep_xwin(x):
    """xwin[p, y, t, b] = xT[28y + 126t + p, b]: the two 126-row K-tiles of
    the 9-row input window for each conv output row y, partition-major and
    y-second so each conv y-chunk loads as one contiguous DMA slice."""
    xT = np.ascontiguousarray(x.reshape(B, 784).T)            # [pix, b]
    p = np.arange(P1)[:, None, None]
    y = np.arange(20)[None, :, None]
    t = np.arange(2)[None, None, :]
    rows = 28 * y + P1 * t + p                                # [126,20,2]
    return np.ascontiguousarray(xT[rows].astype(NPBF))        # [126,20,2,64]


def _prep_wband(conv_w, ch_lo):
    """wband[p, t, (ch,x')] = conv_w[ch_lo+ch, 0, dy, xin-x'] / 8
    where (dy, xin) = divmod(126t + p, 28).  The 1/8 is the uniform
    softmax coupling of routing(u1, 1), folded into the (linear) conv;
    relu(z/8) == relu(z)/8."""
    wb = np.zeros((252, NCH, 20), np.float32)
    cw = conv_w[ch_lo:ch_lo + NCH, 0]                         # [4, 9, 9]
    for idx in range(252):
        dy, xin = divmod(idx, 28)
        for xp in range(max(0, xin - 8), min(20, xin + 1)):
            wb[idx, :, xp] = cw[:, dy, xin - xp] * 0.125
    return np.ascontiguousarray(
        wb.reshape(2, P1, Q).transpose(1, 0, 2).astype(NPBF))  # [126,2,80]


def _prep_w1r(W1, ch_lo):
    """w1r[(y, ch, x'), (j, m, c)] — this core's k-rows of W1 in bf16,
    zero-padded 1600 -> 1664 rows so the stream is 13 full 128-partition
    tiles (zero rows multiply against garbage xk pad rows, contributing 0)."""
    v = W1.reshape(J1, 32, 20, 20, M1 * C1)[:, ch_lo:ch_lo + NCH]
    v = v.transpose(2, 1, 3, 0, 4).reshape(NROW, J1 * M1 * C1)
    out = np.zeros((NBLK * 128, J1 * M1 * C1), NPBF)
    out[:NROW] = v.astype(NPBF)
    return out


def _prep_w2bd(W2):
    """Block-diagonal digit-caps weights, output columns ordered (k, m, j):
    bd[(k',c), (k,m,j)] = W2[j,k,m,c] iff k == k', so u2 = v1_flat @ bd in
    one matmul.  The (k,m,j) order makes every big routing op walk with a
    real stride-1 innermost dim: k-reductions and m-reductions become
    halving adds over contiguous slabs, and the c/v broadcasts put their
    0-stride dim outermost (0-stride innermost measured 1.8x slower)."""
    bd5 = np.zeros((K2, C2, K2, M2, J2), np.float32)
    for k in range(K2):
        bd5[k, :, k, :, :] = W2[:, k, :, :].transpose(2, 1, 0)
    return np.ascontiguousarray(bd5.reshape(K2 * C2, JKM).astype(NPBF))


# ----------------------------------------------------------------------------
# phase A: per-core conv + partial capsule matmul  (SPMD on 8 cores)
# ----------------------------------------------------------------------------

def _build_phase_a():
    nc = bacc.Bacc("TRN2", target_bir_lowering=False, debug=False,
                   num_devices=NCORES)
    xwin_d = nc.dram_tensor("xwin", [P1, 20, 2, B], BF16, kind="ExternalInput")
    wband_d = nc.dram_tensor("wband", [P1, 2, Q], BF16, kind="ExternalInput")
    bias_d = nc.dram_tensor("bias", [Q, 1], F32, kind="ExternalInput")
    w1r_d = nc.dram_tensor("w1r", [NBLK * 128, J1 * M1 * C1], BF16,
                           kind="ExternalInput")
    sp_d = nc.dram_tensor("sp", [128, 32], F32, kind="ExternalOutput")

    with tile.TileContext(nc) as tc:
        with (
            tc.tile_pool(name="const", bufs=1) as const,
            tc.tile_pool(name="w1pool", bufs=1) as w1pool,
            tc.tile_pool(name="xkpool", bufs=1) as xkpool,
            tc.tile_pool(name="cpsum", bufs=1, space="PSUM") as cpsum_pool,
            tc.tile_pool(name="spsum", bufs=1, space="PSUM") as spsum_pool,
        ):
            # conv inputs first on sync; xw in 3 y-chunk DMAs so conv chunk 1
            # starts as soon as its slice lands
            xw = const.tile([P1, 20, 2, B], BF16)
            nc.sync.dma_start(out=xw[:, 0:8, :, :], in_=xwin_d[:, 0:8, :, :])
            wb = const.tile([P1, 2, Q], BF16)
            nc.sync.dma_start(out=wb[:], in_=wband_d[:])
            bias_t = const.tile([Q, 1], F32)
            nc.sync.dma_start(out=bias_t[:], in_=bias_d[:])
            nc.sync.dma_start(out=xw[:, 8:16, :, :], in_=xwin_d[:, 8:16, :, :])
            nc.sync.dma_start(out=xw[:, 16:20, :, :],
                              in_=xwin_d[:, 16:20, :, :])

            # W1 stream: one [128, 13*4KB] tile fed by 4 big DMAs (1.6-2 MB
            # each -- small transfers sit below the SDMA efficiency knee) on
            # gpsimd (SWDGE sustains ~350+ GB/s solo; HWDGE measured ~250).
            # Concurrent queues do NOT share SDMA bandwidth fairly (a
            # megabyte-transfer queue starves a small one ~8:1, measured), so
            # each W1 chunk is write-after-write gated behind a tiny vector
            # copy that depends on xw chunk 2: the W1 stream starts only
            # after the conv input is resident.  Total HBM bytes are
            # unchanged, so the stream still ends at the same time, but conv
            # + repack start ~7 us earlier.  (Program order alone cannot do
            # this: the Tile scheduler reorders queue ops by readiness.)
            w1all = w1pool.tile([128, NBLK, J1 * M1 * C1], BF16)
            # one strided gate copy touching 8 columns of every k-block: each
            # W1 chunk DMA picks up a write-after-write edge on it, and it
            # reads xw chunk 1 -- issued on the same gpsimd queue as the W1
            # DMAs so the whole stream holds until the first conv input lands
            nc.gpsimd.tensor_copy(
                w1all[0:P1, :, 0:8],
                xw[:, 0:8, 0, 0:NBLK].rearrange("p y b -> p b y"))
            for lo, hi in ((0, 4), (4, 8), (8, 12), (12, 13)):
                nc.gpsimd.dma_start(
                    out=w1all[:, lo:hi, :],
                    in_=w1r_d[128 * lo:128 * hi].rearrange(
                        "(i p) c -> p i c", p=128))

            # conv: cps[(ch,x'), (y,b)] += wband_t.T @ xwin[:, y, t, :]
            # 3 N-slices (one PSUM bank each) x 2 K-tiles, t inner.  Each
            # slice is a y-chunk; relu + repack for the chunk follow
            # immediately, split over the two HWDGE queues (scalar/sync), so
            # the repack overlaps the W1 stream instead of trailing it.
            cps = cpsum_pool.tile([Q, 20, B], F32)
            cps_flat = cps[:].rearrange("q y b -> q (y b)")
            xfT = const.tile([Q, 20, B], BF16)
            xk = xkpool.tile([128, NBLK, B], BF16)
            nc.vector.memset(xk[64:128, NBLK - 1, :], 0)  # pad rows 1600-1663
            rp = 0
            rp_eng = [nc.scalar, nc.sync]
            for ylo, yhi in ((0, 8), (8, 16), (16, 20)):
                for t in range(2):
                    nc.tensor.matmul(
                        cps_flat[:, 64 * ylo:64 * yhi], wb[:, t, :],
                        xw[:, ylo:yhi, t, :],
                        start=(t == 0), stop=(t == 1),
                    )
                # fused bias+relu on DVE (keeps ACT idle -> no table load):
                # xfT = max(cps + bias, 0), cast to bf16
                nc.vector.tensor_scalar(xfT[:, ylo:yhi, :],
                                        cps[:, ylo:yhi, :], bias_t[:], 0.0,
                                        op0=ALU.add, op1=ALU.max)
                # repack this chunk's (q, y) rows into the dense 128-row
                # k-blocks (y-major flat row r = 80y + q), SBUF->SBUF
                for y in range(ylo, yhi):
                    r0 = Q * y
                    i0, off = divmod(r0, 128)
                    seg = min(128 - off, Q)
                    rp_eng[rp % 2].dma_start(out=xk[off:off + seg, i0, :],
                                             in_=xfT[0:seg, y, :])
                    rp += 1
                    if seg < Q:
                        rp_eng[rp % 2].dma_start(out=xk[0:Q - seg, i0 + 1, :],
                                                 in_=xfT[seg:Q, y, :])
                        rp += 1

            # stage 2: for each k-block, 4 matmuls in 2 concurrent col-group
            # pairs (M=64 = batch uses half the PE array; tile_position
            # (0,0)/(0,64) runs two at once).  4 separate PSUM banks so each
            # (h,g) accumulation group owns its bank.
            sps = [spsum_pool.tile([128, 512], F32, tag=f"sp{h}{g}",
                                   name=f"sp{h}{g}")
                   for h in range(2) for g in range(2)]
            for i in range(NBLK):
                for h in range(2):
                    for g in range(2):
                        lo = 1024 * h + 512 * g
                        nc.tensor.matmul(
                            sps[2 * h + g][64 * g:64 * g + 64, :],
                            xk[:, i, :], w1all[:, i, lo:lo + 512],
                            start=(i == 0), stop=(i == NBLK - 1),
                            tile_position=(0, 64 * g),
                        )

            # fold c: PSUM [64, (j2, m, c)] -> s_all[(g,b), (h, j2, m)]
            s_all = const.tile([128, 32], F32)
            for h in range(2):
                for g in range(2):
                    nc.vector.reduce_sum(
                        s_all[64 * g:64 * g + 64, 16 * h:16 * h + 16]
                        .rearrange("b (j m) -> b j m", m=M1),
                        sps[2 * h + g][64 * g:64 * g + 64, :]
                        .rearrange("b (j m c) -> b j m c", m=M1, c=C1),
                        axis=AX.X,
                    )
            nc.sync.dma_start(out=sp_d[:], in_=s_all[:])

    nc.compile()
    return nc


# ----------------------------------------------------------------------------
# phase B: squash -> digit caps -> 3-iter routing  (1 core, tiny tensors)
# ----------------------------------------------------------------------------

def _squash(nc, pool, s_ap, j, m, out_tag, out_dtype=BF16, mj=False,
            pre_scale=1.0):
    """v = |s|/(1+|s|^2) * s with the norm over the m-dim.
    Sqrt is the ONLY ACT function phase B uses (softmax exp is a DVE
    polynomial), so there is exactly one ACT_TABLE_LOAD in the kernel.
    n (ACT) and den/rden (DVE) depend only on ss, so they overlap.
    mj=False: s is [B, (j, m)];  mj=True: s is [B, (m, j)] (routing layout,
    keeps j stride-1 innermost for the big broadcast muls).
    pre_scale=beta computes squash(beta * s) from s without ever scaling s:
    n' = sqrt(beta^4 ss) = beta^2 sqrt(ss) and den = 1 + beta^2 ss fold beta
    into existing op immediates, so v = s * n'/den = squash(beta s)."""
    sq = pool.tile([B, j * m], F32, tag=out_tag + "_sq")
    nc.vector.tensor_mul(sq[:], s_ap, s_ap)
    ss = pool.tile([B, j], F32, tag=out_tag + "_ss")
    if mj:
        ssv = sq[:].rearrange("b (m j) -> b j m", j=j)
    else:
        ssv = sq[:].rearrange("b (j m) -> b j m", m=m)
    nc.vector.reduce_sum(ss[:], ssv, axis=AX.X)
    n = pool.tile([B, j], F32, tag=out_tag + "_n")
    nc.scalar.activation(out=n[:], in_=ss[:], func=AF.Sqrt,
                         scale=pre_scale ** 4)
    den = pool.tile([B, j], F32, tag=out_tag + "_den")
    nc.vector.tensor_scalar(den[:], ss[:], pre_scale ** 2, 1.0,
                            op0=ALU.mult, op1=ALU.add)
    rden = pool.tile([B, j], F32, tag=out_tag + "_rden")
    nc.vector.reciprocal(rden[:], den[:])
    f = pool.tile([B, j], F32, tag=out_tag + "_f")
    nc.vector.tensor_mul(f[:], n[:], rden[:])
    v = pool.tile([B, j * m], out_dtype, tag=out_tag)
    if mj:
        nc.vector.tensor_mul(
            v[:].rearrange("b (m j) -> b m j", j=j),
            s_ap.rearrange("b (m j) -> b m j", j=j),
            f[:].to_broadcast([B, j, m]).rearrange("b j m -> b m j"),
        )
    else:
        nc.vector.tensor_mul(
            v[:].rearrange("b (j m) -> b j m", m=m),
            s_ap.rearrange("b (j m) -> b j m", m=m),
            f[:].to_broadcast([B, j, m]),
        )
    return v


def _build_phase_b():
    nc = bacc.Bacc("TRN2", target_bir_lowering=False, debug=False,
                   num_devices=1)
    s_d = nc.dram_tensor("s", [B, JM], F32, kind="ExternalInput")
    w2bd_d = nc.dram_tensor("w2bd", [K2 * C2, JKM], BF16, kind="ExternalInput")
    ident_d = nc.dram_tensor("ident", [B, B], BF16, kind="ExternalInput")
    v2_d = nc.dram_tensor("v2", [B, J2 * M2], F32, kind="ExternalOutput")

    with tile.TileContext(nc) as tc:
        with (
            tc.tile_pool(name="sb", bufs=1) as sb,
            tc.tile_pool(name="ps", bufs=1, space="PSUM") as ps,
        ):
            s_t = sb.tile([B, JM], F32)
            nc.sync.dma_start(out=s_t[:], in_=s_d[:])
            ident_t = sb.tile([B, B], BF16)
            nc.sync.dma_start(out=ident_t[:], in_=ident_d[:])
            w2bd_t = sb.tile([K2 * C2, JKM], BF16)
            nc.sync.dma_start(out=w2bd_t[:], in_=w2bd_d[:])

            # s is [b, (k, c)] (primary caps j1=k, m1=c), norm over c
            v1 = _squash(nc, sb, s_t[:], J1, M1, "v1")

            # v1T = v1.T via PE so it can be the stationary operand
            tp = ps.tile([B, B], BF16, tag="tp")
            nc.tensor.transpose(tp[:], v1[:], ident_t[:])
            v1t = sb.tile([B, B], BF16)
            nc.vector.tensor_copy(v1t[:], tp[:])

            # u2[b, (k,m,j)] = v1_flat @ w2bd, copied once to SBUF bf16 (DVE
            # ops with a PSUM-f32 operand measured ~1.8x slower than SBUF
            # bf16).  k is outermost so k-reductions are halving adds over
            # contiguous slabs; j is stride-1 innermost so every broadcast
            # operand keeps real data innermost.
            up = ps.tile([B, JKM], F32, tag="up")
            for lo, hi in ((0, 512), (512, 1024), (1024, JKM)):
                nc.tensor.matmul(up[:, lo:hi], v1t[:], w2bd_t[:, lo:hi],
                                 start=True, stop=True)
            u2 = sb.tile([B, JKM], BF16)
            nc.vector.tensor_copy(u2[:], up[:])
            u2_kmj = u2[:].rearrange("b (k m j) -> b k m j", m=M2, j=J2)

            bij = sb.tile([B, K2 * J2], F32)
            tmp = sb.tile([B, K2, M2 * J2], BF16)   # m-tree operand
            tmp2 = sb.tile([B, JKM], BF16)          # k-tree operand
            t4k = sb.tile([B, JKM // 2], BF16)
            t2k = sb.tile([B, JKM // 4], BF16)
            t8m = sb.tile([B, K2, M2 * J2 // 2], BF16)
            t4m = sb.tile([B, K2, M2 * J2 // 4], BF16)
            t2m = sb.tile([B, K2, M2 * J2 // 8], BF16)
            s2 = sb.tile([B, M2 * J2], F32)

            def ktree(src, psum=False):  # sum over k (outermost): halving adds
                if psum:
                    # DVE may read only ONE input from PSUM: copy lo half to
                    # SBUF first, then accumulate the hi half from PSUM
                    nc.vector.tensor_copy(t4k[:], src[:, 0:640])
                    nc.vector.tensor_add(t4k[:], t4k[:], src[:, 640:1280])
                else:
                    nc.vector.tensor_add(t4k[:], src[:, 0:640],
                                         src[:, 640:1280])
                nc.vector.tensor_add(t2k[:], t4k[:, 0:320], t4k[:, 320:640])
                nc.vector.tensor_add(s2[:], t2k[:, 0:160], t2k[:, 160:320])

            def mtree(out_kj):  # sum tmp over m (middle): 4 halving adds
                nc.vector.tensor_add(t8m[:], tmp[:, :, 0:80], tmp[:, :, 80:160])
                nc.vector.tensor_add(t4m[:], t8m[:, :, 0:40], t8m[:, :, 40:80])
                nc.vector.tensor_add(t2m[:], t4m[:, :, 0:20], t4m[:, :, 20:40])
                nc.vector.tensor_add(out_kj, t2m[:, :, 0:10], t2m[:, :, 10:20])

            v = None
            for it in range(3):
                if it == 0:
                    # softmax of zeros over j is uniform: s2 = 0.1 * sum_k u2
                    ktree(u2)
                else:
                    # routing logits are ~1e-3, so exp(b) = 1 + b(1 + b/2) to
                    # fp32 accuracy (softmax only needs the ratios); this
                    # keeps exp off ACT so Sqrt never alternates table sets
                    eu = sb.tile([B, K2 * J2], F32, tag="eu")
                    nc.vector.tensor_scalar(eu[:], bij[:], 0.5, 1.0,
                                            op0=ALU.mult, op1=ALU.add)
                    e = sb.tile([B, K2 * J2], F32, tag="e")
                    nc.vector.tensor_mul(e[:], bij[:], eu[:])
                    nc.vector.tensor_scalar_add(e[:], e[:], 1.0)
                    dn = sb.tile([B, K2], F32, tag="dn")
                    nc.vector.reduce_sum(
                        dn[:], e[:].rearrange("b (k j) -> b k j", j=J2),
                        axis=AX.X)
                    rdn = sb.tile([B, K2], F32, tag="rdn")
                    nc.vector.reciprocal(rdn[:], dn[:])
                    c = sb.tile([B, K2 * J2], BF16, tag="c")
                    nc.vector.tensor_mul(
                        c[:].rearrange("b (k j) -> b k j", j=J2),
                        e[:].rearrange("b (k j) -> b k j", j=J2),
                        rdn[:].to_broadcast([B, K2, J2]),
                    )
                    # s2[b,m,j] = sum_k c[b,k,j] * u2[b,k,m,j]
                    nc.vector.tensor_mul(
                        tmp2[:].rearrange("b (k m j) -> b k m j",
                                          m=M2, j=J2),
                        u2_kmj,
                        c[:].rearrange("b (k j) -> b k j", j=J2)
                        .to_broadcast([B, K2, J2, M2])
                        .rearrange("b k j m -> b k m j"),
                    )
                    ktree(tmp2)
                v = _squash(nc, sb, s2[:], J2, M2, "v", mj=True,
                            out_dtype=(F32 if it == 2 else BF16),
                            pre_scale=(1.0 / J2 if it == 0 else 1.0))
                if it < 2:
                    # bij[b,k,j] += sum_m u2[b,k,m,j] * v[b,m,j]
                    nc.vector.tensor_mul(
                        tmp[:].rearrange("b k (m j) -> b k m j", j=J2),
                        u2_kmj,
                        v[:].rearrange("b (m j) -> b m j", j=J2)
                        .to_broadcast([B, M2, J2, K2])
                        .rearrange("b m j k -> b k m j"),
                    )
                    if it == 0:
                        mtree(bij[:].rearrange("b (k j) -> b k j", j=J2))
                    else:
                        bupd = sb.tile([B, K2 * J2], F32, tag="bupd")
                        mtree(bupd[:].rearrange("b (k j) -> b k j", j=J2))
                        nc.vector.tensor_add(bij[:], bij[:], bupd[:])

            # v is [b, (m, j)]; the host transposes to [b, (j, m)]
            nc.sync.dma_start(out=v2_d[:], in_=v[:])

    nc.compile()
    return nc


# ----------------------------------------------------------------------------
# entry point
# ----------------------------------------------------------------------------

LAST_RESULTS = []  # [phase_a BassKernelResults, phase_b BassKernelResults]


def kernel(x, conv_w, conv_b, W1, W2):
    x = np.ascontiguousarray(np.asarray(x, np.float32))
    conv_w = np.asarray(conv_w, np.float32)
    conv_b = np.asarray(conv_b, np.float32)
    W1 = np.asarray(W1, np.float32)
    W2 = np.asarray(W2, np.float32)

    if "a" not in _CACHE:
        _CACHE["a"] = _build_phase_a()
        _CACHE["b"] = _build_phase_b()
    nca, ncb = _CACHE["a"], _CACHE["b"]

    xwin = _prep_xwin(x)
    in_maps = []
    for i in range(NCORES):
        in_maps.append({
            "xwin": xwin,
            "wband": _prep_wband(conv_w, NCH * i),
            "bias": np.ascontiguousarray(
                np.repeat(conv_b[NCH * i:NCH * i + NCH] * 0.125, 20)
            ).reshape(Q, 1),
            "w1r": _prep_w1r(W1, NCH * i),
        })

    ra = run_bass_kernel_spmd(nca, in_maps, list(range(NCORES)))
    s128 = np.sum([r["sp"] for r in ra.results], axis=0, dtype=np.float32)
    # [(g,b), (h,j2,m)] -> [b, (j=4h+2g+j2, m)]
    s = np.ascontiguousarray(
        s128.reshape(2, B, 2, 2, M1).transpose(1, 2, 0, 3, 4).reshape(B, JM))

    rb = run_bass_kernel_spmd(
        ncb,
        [{"s": s, "w2bd": _prep_w2bd(W2),
          "ident": np.eye(B, dtype=NPBF)}],
        [0],
    )
    LAST_RESULTS[:] = [ra, rb]
    # v2 comes back in (m, j) order; transpose to [b, j, m]
    return np.ascontiguousarray(
        rb.results[0]["v2"].reshape(B, M2, J2).transpose(0, 2, 1))
